# revision 28
# baseline (speedup 1.0000x reference)
"""Trainium2 Bass kernel for the CIN (xDeepFM) block.

inputs [2048,39,16] f32, W0 [1521,128], W1 [4992,128] -> out [2048,256] f32.
Data-parallel over the batch axis across 8 NeuronCores; weights replicated.

Per-core pipeline (all matmuls bf16, fp32 PSUM accumulation):
  sq-build : 6 K-tiles of strict-pair (m<n) sums via summed-selection matmuls;
             squares applied during the PSUM evacuation (Act/DVE alternating).
             Diagonal terms folded into per-field x^2 correction weights; the
             x^2 tensor itself is built on GpSimd from SBUF (no PSUM port
             needed), keeping it off the Act/DVE evacuation budget.
  X1       : per 128-row chunk, 6 sq-tile matmuls + 1 x^2 matmul accumulate
             the layer-0 output [r, o].
  H        : per-batch Gram H[o, (b8, m)] via one K=128 matmul per chunk
             against a block-diagonal X0 whose extra one-hot columns also
             yield the out1 d-sums.
  out2     : flipped orientation (W1 m-slice stationary, H batch columns
             streamed) -> transposed output [o2, b]; out emitted transposed
             and fixed up on the host.
"""


import ml_dtypes
import numpy as np

BF16 = ml_dtypes.bfloat16

B, M0, D = 2048, 39, 16
C0, C1 = 128, 128
NCORES = 8
BL = B // NCORES          # 256 batches per core
R = BL * D                # 4096 rows per core
NPAIR = (M0 * (M0 - 1)) // 2   # 741 strict pairs
KT = 6                          # 6 K-tiles of 128
KPAD = KT * 128                 # 768
RC = 512                        # r-chunk for sq-build
NRC = R // RC                   # 8
NCHUNK = R // 128               # 32 chunks of (8 b x 16 d)
BPC = 128 // D                  # 8 batches per 128-row chunk
HW_ = BPC * (M0 + 1)            # 320


def host_constants(W0, W1):
    """Core-independent prepped tensors.

    Square trick (strict pairs only): x_m*x_n = 0.5*(x_m+x_n)^2
    - 0.5*x_m^2 - 0.5*x_n^2; the x^2 corrections and the diagonal W0
    terms are carried by w0x against the separately-built x^2 tensor.
    """
    pairs = [(m, n) for m in range(M0) for n in range(m + 1, M0)]
    assert len(pairs) == NPAIR and NPAIR <= KPAD

    selsum = np.zeros((128, KT, 128), dtype=np.float32)
    for p, (m, n) in enumerate(pairs):
        t, q = divmod(p, 128)
        selsum[m, t, q] += 1.0
        selsum[n, t, q] += 1.0

    W0r = W0.reshape(M0, M0, C0)
    w0h = np.zeros((KPAD, C0), dtype=np.float32)
    for p, (m, n) in enumerate(pairs):
        w0h[p] = 0.5 * (W0r[m, n] + W0r[n, m])
    w0h_kt = w0h.reshape(KT, 128, C0).transpose(1, 0, 2).copy()

    # x^2 weights: diagonal W0 plus the -0.5*(x_m^2+x_n^2) corrections
    w0x = np.zeros((128, C0), dtype=np.float32)
    for m in range(M0):
        w0x[m] = W0r[m, m]
    for p, (m, n) in enumerate(pairs):
        w0x[m] -= w0h[p]
        w0x[n] -= w0h[p]

    # w1sb[o, m, o2] = W1[m*C0+o, o2]
    w1sb = W1.reshape(M0, C0, C1).transpose(1, 0, 2).copy()

    return dict(
        selsum=np.ascontiguousarray(selsum.astype(BF16)),
        w0h=np.ascontiguousarray(w0h_kt.astype(BF16)),
        w0x=np.ascontiguousarray(w0x.astype(BF16)),
        w1sb=np.ascontiguousarray(w1sb.astype(BF16)),
    )


def host_core_inputs(x_c):
    """Per-core prepped tensors from the [BL, M0, D] input shard."""
    xdT = np.zeros((128, R), dtype=np.float32)
    # xdT[m, b*D+d] = x[b, m, d]
    xdT[:M0] = x_c.transpose(1, 0, 2).reshape(M0, R)
    # full block-diagonal Gram operand, zero-padded on the host:
    # xT2z[(b8, d), ch, b8'*(M0+1)+m] = x[ch*8+b8, m, d] if b8 == b8' else 0
    xtt = x_c.reshape(NCHUNK, BPC, M0, D).transpose(1, 3, 0, 2)  # [b8, d, ch, m]
    xT2z = np.zeros((BPC, D, NCHUNK, BPC, M0 + 1), dtype=np.float32)
    for b8 in range(BPC):
        xT2z[b8, :, :, b8, :M0] = xtt[b8]
        xT2z[b8, :, :, b8, M0] = 1.0  # d-sum -> out1^T column
    xT2z = xT2z.reshape(128, NCHUNK, HW_)
    return dict(
        xdT=np.ascontiguousarray(xdT.astype(BF16)),
        xt=np.ascontiguousarray(xT2z.astype(BF16)),
    )


def split_sync_waits(nc):
    """Rewrite every instruction carrying >1 sync wait: keep the first wait,
    hoist the rest onto same-engine NoOps inserted immediately before it."""
    import concourse.mybir as mybir

    counter = [0]
    for f in nc.m.functions:
        for bb in f.blocks:
            new_list = []
            changed = False
            for inst in bb.instructions:
                si = inst.sync_info
                waits = list(si.on_wait) if si is not None else []
                if len(waits) > 1:
                    changed = True
                    for w in waits[:-1]:
                        counter[0] += 1
                        nop = mybir.InstNoOp(
                            name=f"WSPLIT-{counter[0]}", ins=[], outs=[]
                        )
                        nop.engine = inst.engine
                        nop.sync_info = mybir.SyncInfo(on_wait=[w], on_update=[])
                        new_list.append(nop)
                    si.on_wait = waits[-1:]
                new_list.append(inst)
            if changed:
                bb.instructions = new_list
    return counter[0]


def build_program(reps=1, split_waits=True, loop_reps=None, cfg=None):
    """loop_reps: if set, wrap the whole body in a tc.For_i hardware loop with
    that trip count (for slope-based HW timing)."""
    import contextlib

    cfg = cfg or {}
    SQ_PS_BUFS = cfg.get("sq_ps", 2)
    X1_PS_BUFS = cfg.get("x1_ps", 2)
    H_PS_BUFS = cfg.get("h_ps", 2)
    SQ_BUFS = cfg.get("sq", 2)
    XSPLIT = cfg.get("xsplit", 4)
    TSPLIT = cfg.get("tsplit", 4)

    import concourse.bass as bass
    import concourse.mybir as mybir
    import concourse.tile as tile

    f32 = mybir.dt.float32
    bf16 = mybir.dt.bfloat16

    nc = bass.Bass("TRN2", target_bir_lowering=False, debug=False)
    d_xdT = nc.dram_tensor("xdT", [128, R], bf16, kind="ExternalInput")
    d_xt = nc.dram_tensor("xt", [128, NCHUNK, HW_], bf16, kind="ExternalInput")
    d_sel = nc.dram_tensor("selsum", [128, KT, 128], bf16, kind="ExternalInput")
    d_w0 = nc.dram_tensor("w0h", [128, KT, C0], bf16, kind="ExternalInput")
    d_w0x = nc.dram_tensor("w0x", [128, C0], bf16, kind="ExternalInput")
    d_w1 = nc.dram_tensor("w1sb", [128, M0, C1], bf16, kind="ExternalInput")
    d_out = nc.dram_tensor("out", [128, 2, BL], f32, kind="ExternalOutput")

    with tile.TileContext(nc) as tc:
        with (
            tc.tile_pool(name="const", bufs=1) as cpool,
            tc.tile_pool(name="xsqp", bufs=1) as xsqpool,
            tc.tile_pool(name="sq", bufs=SQ_BUFS) as sqpool,
            tc.tile_pool(name="x1sb", bufs=1) as x1pool,
            tc.tile_pool(name="hsb", bufs=1) as hpool,
            tc.tile_pool(name="outp", bufs=1) as opool,
            tc.tile_pool(name="ps_sq", bufs=SQ_PS_BUFS, space="PSUM") as ps_sq,
            tc.tile_pool(name="ps_x1", bufs=X1_PS_BUFS, space="PSUM") as ps_x1,
            tc.tile_pool(name="ps_h", bufs=H_PS_BUFS, space="PSUM") as ps_h,
        ):
            loop_cm = (
                tc.For_i(
                    0,
                    loop_reps,
                    1,
                    hint_engines=(
                        mybir.EngineType.PE,
                        mybir.EngineType.Activation,
                        mybir.EngineType.DVE,
                        mybir.EngineType.SP,
                        mybir.EngineType.Pool,
                    ),
                )
                if loop_reps is not None
                else contextlib.nullcontext()
            )
            with loop_cm:
                for _rep in range(reps):
                    xdT = cpool.tile([128, R], bf16, tag="xdT")
                    sel = cpool.tile([128, KT, 128], bf16, tag="sel")
                    w0 = cpool.tile([128, KT, C0], bf16, tag="w0")
                    w0x = cpool.tile([128, C0], bf16, tag="w0x")
                    w1 = cpool.tile([128, M0, C1], bf16, tag="w1")
                    xT2z = cpool.tile([128, NCHUNK, HW_], bf16, tag="xT2z")
                    xsq = xsqpool.tile([128, R], bf16, tag="xsq")
                    # first matmul's operands first; spread the bulk across
                    # the SP/Act HWDGE queues and the GpSimd SWDGE queue so
                    # transfers run in parallel
                    spread = cfg.get("dma_spread", True)
                    q_xt = nc.scalar if spread else nc.sync
                    q_w1 = nc.scalar if spread else nc.sync
                    nc.sync.dma_start(xdT[:, : R // XSPLIT], d_xdT[:, : R // XSPLIT])
                    nc.sync.dma_start(sel[:], d_sel[:, :, :])
                    nc.sync.dma_start(w0[:], d_w0[:, :, :])
                    nc.sync.dma_start(w0x[:], d_w0x[:, :])
                    for q in range(1, XSPLIT):
                        nc.sync.dma_start(
                            xdT[:, q * (R // XSPLIT) : (q + 1) * (R // XSPLIT)],
                            d_xdT[:, q * (R // XSPLIT) : (q + 1) * (R // XSPLIT)],
                        )
                    for q in range(TSPLIT):
                        cs = slice(q * (NCHUNK // TSPLIT), (q + 1) * (NCHUNK // TSPLIT))
                        q_xt.dma_start(xT2z[:, cs, :], d_xt[:, cs, :])
                    q_w1.dma_start(w1[:], d_w1[:, :, :])

                    # x^2 tensor (SBUF->SBUF so any vector engine qualifies).
                    # Full 128 partitions: rows >= M0 are host-zeroed in xdT,
                    # so the X1 matmul below keeps a 128x128 PE tile config.
                    xsq_eng = nc.gpsimd if cfg.get("xsq_pool", True) else nc.vector
                    for k in range(2 * XSPLIT):
                        xs = slice(k * (R // (2 * XSPLIT)), (k + 1) * (R // (2 * XSPLIT)))
                        xsq_eng.tensor_mul(xsq[:, xs], xdT[:, xs], xdT[:, xs])

                    x1sb = x1pool.tile([128, NCHUNK, C0], bf16, tag="x1sb")
                    # H stored [o, m, b] so out2 streams contiguous batch
                    # columns and out1^T is a contiguous slice
                    hsb = hpool.tile([128, M0 + 1, BL], bf16, tag="hsb")
                    outsb = opool.tile([128, 2, BL], f32, tag="outsb")

                    def emit_half(bt):
                        # out2 transposed: W1 m-slice stationary, H batch
                        # columns streamed -> psum [o2, 128 b]; borrows the
                        # x1 psum pool (same shape/tag)
                        MEMIT = cfg.get("o2_mtrunc", M0)
                        flip = cfg.get("o2_orient", "flip") == "flip"
                        o2_ps = ps_x1.tile([128, C0], f32, tag="x1")
                        for m in range(MEMIT):
                            a = w1[:, m, :]
                            b = hsb[:, m, bt * 128 : (bt + 1) * 128]
                            if not flip:
                                a, b = b, a
                            nc.tensor.matmul(
                                o2_ps[:],
                                a,
                                b,
                                start=(m == 0),
                                stop=(m == MEMIT - 1),
                            )
                        if cfg.get("skip_o2evac", False):
                            return
                        # DVE for both halves: a scalar.copy would churn the
                        # Act activation table between Square and Copy
                        nc.vector.tensor_copy(
                            outsb[:, 1, bt * 128 : (bt + 1) * 128], o2_ps[:]
                        )
                        if cfg.get("skip_o2dma", False):
                            return
                        nc.sync.dma_start(
                            d_out[:, 1, bt * 128 : (bt + 1) * 128],
                            outsb[:, 1, bt * 128 : (bt + 1) * 128],
                        )

                    for rc in range(NRC):
                        rsl = slice(rc * RC, (rc + 1) * RC)
                        sq6 = sqpool.tile([128, KT, RC], bf16, tag="sq6")
                        for j in range(3):
                            sum_ps = ps_sq.tile([128, 2, RC], f32, tag="sum")
                            for i in (0, 1):
                                nc.tensor.matmul(
                                    sum_ps[:, i, :],
                                    sel[:, 2 * j + i, :],
                                    xdT[:, rsl],
                                    start=True,
                                    stop=True,
                                )
                            # square during evacuation (Act only: TensorTensor
                            # may read just one PSUM input, so DVE can't square
                            # from PSUM; DVE carries the x1/hsb copies instead)
                            nc.scalar.square(sq6[:, 2 * j : 2 * j + 2, :], sum_ps[:])
                        for rs in range(RC // 128):
                            ch = rc * (RC // 128) + rs
                            csl = slice(rs * 128, (rs + 1) * 128)
                            x1_ps = ps_x1.tile([128, C0], f32, tag="x1")
                            for t in range(KT):
                                nc.tensor.matmul(
                                    x1_ps[:],
                                    sq6[:, t, csl],
                                    w0[:, t, :],
                                    start=(t == 0),
                                    stop=False,
                                )
                            nc.tensor.matmul(
                                x1_ps[:],
                                xsq[:, ch * 128 : (ch + 1) * 128],
                                w0x[:],
                                start=False,
                                stop=True,
                            )
                            nc.vector.tensor_copy(x1sb[:, ch, :], x1_ps[:])
                            # all 8 per-batch Grams of this chunk in one
                            # K=128 matmul vs the block-diagonal x
                            h_ps = ps_h.tile([128, HW_], f32, tag="h")
                            nc.tensor.matmul(
                                h_ps[:],
                                x1sb[:, ch, :],
                                xT2z[:, ch, :],
                                start=True,
                                stop=True,
                            )
                            nc.vector.tensor_copy(
                                hsb[:, :, ch * BPC : (ch + 1) * BPC].rearrange(
                                    "p m b -> p b m"
                                ),
                                h_ps[:],
                            )
                            if (
                                ch == 15
                                and not cfg.get("skip_emit", False)
                                and not cfg.get("emit_at_end", False)
                            ):
                                emit_half(0)
                    if not cfg.get("skip_emit", False):
                        if cfg.get("emit_at_end", False):
                            emit_half(0)
                        emit_half(1)
                    # out1^T: d-sum row gathered straight from hsb
                    out1_eng = nc.gpsimd if cfg.get("out1_pool", True) else nc.vector
                    out1_eng.tensor_copy(outsb[:, 0, :], hsb[:, M0, :])
                    nc.sync.dma_start(d_out[:, 0, :], outsb[:, 0, :])

    if split_waits:
        split_sync_waits(nc)
    return nc


def make_in_maps(inputs, W0, W1):
    consts = host_constants(np.asarray(W0), np.asarray(W1))
    in_maps = []
    for c in range(NCORES):
        x_c = np.ascontiguousarray(np.asarray(inputs)[c * BL : (c + 1) * BL])
        m = dict(consts)
        m.update(host_core_inputs(x_c))
        in_maps.append(m)
    return in_maps


_KERNEL_CACHE = {}


def kernel(inputs, W0, W1):
    inputs = np.ascontiguousarray(np.asarray(inputs, dtype=np.float32))
    W0 = np.ascontiguousarray(np.asarray(W0, dtype=np.float32))
    W1 = np.ascontiguousarray(np.asarray(W1, dtype=np.float32))
    assert inputs.shape == (B, M0, D) and W0.shape == (M0 * M0, C0)
    assert W1.shape == (M0 * C0, C1)

    if "nc" not in _KERNEL_CACHE:
        _KERNEL_CACHE["nc"] = build_program()
    nc = _KERNEL_CACHE["nc"]

    in_maps = make_in_maps(inputs, W0, W1)

    from concourse.bass_utils import run_bass_kernel_spmd

    res = run_bass_kernel_spmd(nc, in_maps, core_ids=list(range(NCORES)))
    # out is emitted transposed per core: [128 (o), 2 (layer), BL (b)]
    parts = []
    for c in range(NCORES):
        r = np.asarray(res.results[c]["out"])
        parts.append(r.transpose(2, 1, 0).reshape(BL, C0 + C1))
    return np.ascontiguousarray(np.concatenate(parts, axis=0).astype(np.float32))
